# revision 47
# baseline (speedup 1.0000x reference)
"""Bass/Trainium2 kernel for the decomposed LocallyConnected2d layer.

out[b,o,i,j] = sum_{c,k} x[b, c, i+di, j+dj] * w[o, c, i, j, k] + bias[o,i,j]
with k = di*3 + dj (3x3 kernel, stride 1).

Strategy: shard over output rows i across 8 cores (4 rows each). Per output
location (i,j) the contraction (di,c,dj)=288 is split into 3 chunks of 96 =
(di,c), chunked over dj; each chunk is one matmul lhsT=[96,64] rhs=[96,128]
accumulating into PSUM [64 o, 128 b]. Even/odd j use PE column groups 0/1
(tile_position) so two locations' matmuls overlap in the array.

All weight chunks ship as unscaled fp8-e4m3 (measured rel-err 1.74e-2 on
the real seed vs the 2e-2 gate; the PE runs fp8-lhsT x fp16-rhs at ~1/3
the fp16 rate, but the DMA savings win because the kernel must finish
before the ~35us activity-governor clamp). The bias stays exact fp16 and
is added during the PSUM->SBUF copies on the vector engine:
tensor_add(out_fp16, psum, bias) with the [128, NGRP] bias slice
broadcast over the batch dim via a stride-0 AP. This keeps the bias off
the PE entirely - a previous revision spent ~13us of PE time on
selector-bias matmuls, and the PE paces the rows. (gpsimd.tensor_add
breaks the neuronxcc compile; keep the copies on vector.)

DMA design: ~7.8 MB/core against a ~260 GB/s effective DMA ceiling, and an
activity governor that halves the core's speed once it has been busy for
~30us - so the whole kernel must land before ~36us. Inputs stream on the
two HWDGE queues (which drain FIFO, and split a DMA across the 16 DMA
engines only when the outermost AP dim is a multiple of 16 - hence 96/128
partition bulk DMAs): per-row x slabs (0.84MB, 8.7KB runs) on scalar,
per-row fp8 weights (0.59MB, 6KB runs) on sync behind the tiny
selector+bias DMAs. The sync ring drains ~9us before scalar, so the
per-row output DMAs appended to it start almost immediately as each row's
copies finish, overlapping the remaining x stream. (The gpsimd SWDGE
queue is no help here: its packets are only serviced once the HW queues
run dry.) 14 DMAs total, tiny ones first so DMA-completion-sem reuse
never chains behind a live transfer.
"""

import sys

for _p in ("/opt/trn_rl_repo", "/root/.axon_site/_ro/trn_rl_repo"):
    if _p not in sys.path:
        sys.path.append(_p)

import numpy as np

B = 128
C_IN = 32
C_OUT = 64
OH = OW = 32
KH = KW = 3
H = W = 34
N_CORES = 8
RPC = OH // N_CORES          # output rows per core = 4
HALO = RPC + KH - 1          # x rows per core = 6
NPAIR = OW // 2              # j-pairs per row = 16
NGRP = 4                     # j-pairs per psum group
GRPS = NPAIR // NGRP         # psum groups per row = 4

_prog_cache = {}


def _build_program():
    import concourse.tile as tile
    from concourse import bacc, mybir

    f16 = mybir.dt.float16
    f8 = mybir.dt.float8e4
    f32 = mybir.dt.float32

    nc = bacc.Bacc("TRN2", target_bir_lowering=False, debug=False,
                   num_devices=N_CORES)

    # Per-core DRAM I/O (host pre-sharded / pre-transposed):
    #   xs  [96, i=4, w=34, b=128] f16   partition p=di*32+c = x row r0+i+di
    #   w   [96, i=4, j=32, dj=3, o=64] f8   all chunks, scaled x256
    #   bias[4, i=4, g=4, par=2, o=64] f16   row k = j-pair slot, scaled x256
    #   sel [4, 4*128] f16               one-hot: sel[k, (pig, b)] = (pig==k)
    #   out [p2=128 (par*64+o), i=4, jh=16, b=128] f16 ; j = 2*jh + par
    xs_in = nc.dram_tensor("xs", [96, RPC, W, B], f16,
                           kind="ExternalInput").ap()
    w_in = nc.dram_tensor("w", [96, RPC, OW, KW, C_OUT], f8,
                          kind="ExternalInput").ap()
    # bias laid out to match the psum/output partition order p = par*64+o,
    # free = (i, g, k): one tiny DMA, read broadcast over b in the copies.
    bias_in = nc.dram_tensor("bias2", [128, RPC, GRPS, NGRP], f16,
                             kind="ExternalInput").ap()
    out = nc.dram_tensor("out", [128, RPC, NPAIR, B], f16,
                         kind="ExternalOutput").ap()

    with tile.TileContext(nc) as tc:
        with (
            tc.tile_pool(name="xpool", bufs=1) as xpool,
            tc.tile_pool(name="wpool", bufs=1) as wpool,
            tc.tile_pool(name="opool", bufs=4) as opool,
            tc.tile_pool(name="pspool", bufs=6, space="PSUM") as pspool,
        ):
            xr = [xpool.tile([96, W, B], f16, tag=f"x{i}", name=f"x{i}")
                  for i in range(RPC)]
            w01 = wpool.tile([96, 2, OW, KW, C_OUT], f8, tag="w01")
            w23 = wpool.tile([96, 2, OW, KW, C_OUT], f8, tag="w23")
            biast = wpool.tile([128, RPC, GRPS, NGRP], f16, tag="biast")

            # 11 DMAs against a ~8-deep DMA-completion-sem pool: a recycled
            # sem makes every reader of the OLD owner's data wait for the
            # NEW owner's transfer (a 14-DMA variant chained rows 2-3's
            # bias matmuls behind rows 0-1's output writes). Emission order
            # ensures only the late out DMAs recycle sems (of selbias/x0/
            # x1, whose readers all executed long before) - harmless.
            # Tiny selector+bias first, then bulk in consumption order:
            # x per-row on scalar, w row-pairs on sync. The sync ring
            # drains first so the per-row output DMAs appended to it start
            # almost immediately once each row's copies finish.
            nc.sync.dma_start(biast[:], bias_in[:])
            for i in range(RPC):
                nc.scalar.dma_start(xr[i][:], xs_in[:, i])
            nc.sync.dma_start(w01[:], w_in[:, 0:2])
            nc.sync.dma_start(w23[:], w_in[:, 2:4])
            w_h = [w01, w23]

            for i in range(RPC):
                orow = opool.tile([128, NPAIR, B], f16, tag=f"o{i}",
                                  name=f"o{i}")
                wt = w_h[i // 2]
                ii = i % 2
                xt = xr[i]
                for g in range(GRPS):
                    ps = pspool.tile([128, NGRP, B], f32)
                    for pig in range(NGRP):
                        for par in range(2):
                            j = 2 * (NGRP * g + pig) + par
                            pslice = ps[64 * par:64 * par + 64, pig, :]
                            tp = (0, 64 * par)
                            nc.tensor.matmul(pslice, wt[:, ii, j, 0, :],
                                             xt[:, j, :],
                                             start=True, stop=False,
                                             tile_position=tp)
                            nc.tensor.matmul(pslice, wt[:, ii, j, 1, :],
                                             xt[:, j + 1, :],
                                             start=False, stop=False,
                                             tile_position=tp)
                            nc.tensor.matmul(pslice, wt[:, ii, j, 2, :],
                                             xt[:, j + 2, :],
                                             start=False, stop=True,
                                             tile_position=tp)
                    # bias enters here, broadcast over b - off the PE's
                    # critical path (the PE paces the rows; 32 selector
                    # matmuls cost it ~13us in the previous revision).
                    dst = orow[:, NGRP * g:NGRP * (g + 1), :]
                    bias_ap = biast[:, i, g, :].unsqueeze(2).broadcast_to(
                        [128, NGRP, B])
                    nc.vector.tensor_add(dst, ps[:], bias_ap)
                nc.sync.dma_start(out[:, i], orow[:])

    nc.compile()
    return nc


def _host_prep(x, weight, bias):
    """Full fp32 inputs -> list of per-core input dicts."""
    import ml_dtypes
    f8 = ml_dtypes.float8_e4m3

    # x: (B, C, H, W) -> (C, H, W, B) fp16
    x_t = np.ascontiguousarray(x.transpose(1, 2, 3, 0)).astype(np.float16)
    # w: (O, C, I, J, K) with K=(di*3+dj) -> [(di*32+c)=96, I, J, dj, O]
    # unscaled: the bias-add happens post-accumulation, so psum must hold
    # the true (unscaled) partial sums. Unscaled fp8 w measures 1.69e-2.
    w_r = weight.reshape(C_OUT, C_IN, OH, OW, KH, KW)
    w_t = w_r.transpose(4, 1, 2, 3, 5, 0)          # (di, c, I, J, dj, O)
    w_full = w_t.reshape(96, OH, OW, KW, C_OUT).astype(f8)
    # bias: (O, I, J) with j = 8g + 2k + par -> [(par*64+o), I, g, k]
    b_t = bias.reshape(C_OUT, OH, GRPS, NGRP, 2)   # (o, I, g, k, par)
    b_t = np.ascontiguousarray(b_t.transpose(4, 0, 1, 2, 3))  # (par,o,I,g,k)
    b_t = b_t.reshape(128, OH, GRPS, NGRP).astype(np.float16)

    in_maps = []
    for m in range(N_CORES):
        r0 = m * RPC
        xs = np.empty((96, RPC, W, B), np.float16)
        xsv = xs.reshape(KH, C_IN, RPC, W, B)
        for di in range(KH):
            xsv[di] = x_t[:, r0 + di:r0 + di + RPC]
        in_maps.append({
            "xs": xs,
            "w": np.ascontiguousarray(w_full[:, r0:r0 + RPC]),
            "bias2": np.ascontiguousarray(b_t[:, r0:r0 + RPC]),
        })
    return in_maps


def _gather(results):
    out_full = np.empty((B, C_OUT, OH, OW), np.float32)
    for m in range(N_CORES):
        r = results[m]["out"].astype(np.float32)          # (128, 4, 16, 128)
        r = r.reshape(2, C_OUT, RPC, NPAIR, B)            # par,o,i,jh,b
        r = r.transpose(4, 1, 2, 3, 0)                    # b,o,i,jh,par
        out_full[:, :, m * RPC:(m + 1) * RPC, :] = r.reshape(B, C_OUT, RPC, OW)
    return out_full


def kernel(x, weight, bias, _trace=False):
    from concourse.bass_utils import run_bass_kernel_spmd

    if "nc" not in _prog_cache:
        _prog_cache["nc"] = _build_program()
    nc = _prog_cache["nc"]

    in_maps = _host_prep(np.asarray(x), np.asarray(weight), np.asarray(bias))
    res = run_bass_kernel_spmd(nc, in_maps, core_ids=list(range(N_CORES)),
                               trace=_trace)
    out = _gather(res.results)
    if _trace:
        _prog_cache["last_result"] = res
    return out


# revision 50
# speedup vs baseline: 1.0379x; 1.0379x over previous
"""Bass/Trainium2 kernel for the decomposed LocallyConnected2d layer.

out[b,o,i,j] = sum_{c,k} x[b, c, i+di, j+dj] * w[o, c, i, j, k] + bias[o,i,j]
with k = di*3 + dj (3x3 kernel, stride 1).

Strategy: shard over output rows i across 8 cores (4 rows each). Per output
location (i,j) the contraction (di,c,dj)=288 is split into 3 chunks of 96 =
(di,c), chunked over dj; each chunk is one matmul lhsT=[96,64] rhs=[96,128]
accumulating into PSUM [64 o, 128 b]. Even/odd j use PE column groups 0/1
(tile_position) so two locations' matmuls overlap in the array.

All weight chunks ship as unscaled fp8-e4m3 (measured rel-err 1.74e-2 on
the real seed vs the 2e-2 gate; the PE runs fp8-lhsT x fp16-rhs at ~1/3
the fp16 rate, but the DMA savings win because the kernel must finish
before the ~35us activity-governor clamp). The bias stays exact fp16 and
is added during the PSUM->SBUF copies on the vector engine:
tensor_add(out_fp16, psum, bias) with the [128, NGRP] bias slice
broadcast over the batch dim via a stride-0 AP. This keeps the bias off
the PE entirely - a previous revision spent ~13us of PE time on
selector-bias matmuls, and the PE paces the rows. (gpsimd.tensor_add
breaks the neuronxcc compile; keep the copies on vector.)

DMA design: ~7.8 MB/core against a ~260 GB/s effective DMA ceiling, and an
activity governor that halves the core's speed once it has been busy for
~30us - so the whole kernel must land before ~36us. Inputs stream on the
two HWDGE queues (which drain FIFO, and split a DMA across the 16 DMA
engines only when the outermost AP dim is a multiple of 16 - hence 96/128
partition bulk DMAs): per-row x slabs (0.84MB, 8.7KB runs) on scalar,
per-row fp8 weights (0.59MB, 6KB runs) on sync behind the tiny
selector+bias DMAs. The sync ring drains ~9us before scalar, so the
per-row output DMAs appended to it start almost immediately as each row's
copies finish, overlapping the remaining x stream. (The gpsimd SWDGE
queue is no help here: its packets are only serviced once the HW queues
run dry.) 14 DMAs total, tiny ones first so DMA-completion-sem reuse
never chains behind a live transfer.
"""

import sys

for _p in ("/opt/trn_rl_repo", "/root/.axon_site/_ro/trn_rl_repo"):
    if _p not in sys.path:
        sys.path.append(_p)

import numpy as np

B = 128
C_IN = 32
C_OUT = 64
OH = OW = 32
KH = KW = 3
H = W = 34
N_CORES = 8
RPC = OH // N_CORES          # output rows per core = 4
HALO = RPC + KH - 1          # x rows per core = 6
NPAIR = OW // 2              # j-pairs per row = 16
NGRP = 4                     # j-pairs per psum group
GRPS = NPAIR // NGRP         # psum groups per row = 4

_prog_cache = {}


def _build_program():
    import concourse.tile as tile
    from concourse import bacc, mybir

    f16 = mybir.dt.float16
    f8 = mybir.dt.float8e4
    f32 = mybir.dt.float32

    nc = bacc.Bacc("TRN2", target_bir_lowering=False, debug=False,
                   num_devices=N_CORES)

    # Per-core DRAM I/O (host pre-sharded / pre-transposed):
    #   xs  [96, i=4, w=34, b=128] f16   partition p=di*32+c = x row r0+i+di
    #   w   [96, i=4, j=32, dj=3, o=64] f8   all chunks, scaled x256
    #   bias[4, i=4, g=4, par=2, o=64] f16   row k = j-pair slot, scaled x256
    #   sel [4, 4*128] f16               one-hot: sel[k, (pig, b)] = (pig==k)
    #   out [p2=128 (par*64+o), i=4, jh=16, b=128] f16 ; j = 2*jh + par
    xs_in = nc.dram_tensor("xs", [96, RPC, W, B], f16,
                           kind="ExternalInput").ap()
    w_in = nc.dram_tensor("w", [96, RPC, OW, KW, C_OUT], f8,
                          kind="ExternalInput").ap()
    # bias laid out to match the psum/output partition order p = par*64+o,
    # free = (i, g, k): one tiny DMA, read broadcast over b in the copies.
    bias_in = nc.dram_tensor("bias2", [128, RPC, GRPS, NGRP], f16,
                             kind="ExternalInput").ap()
    out = nc.dram_tensor("out", [128, RPC, NPAIR, B], f16,
                         kind="ExternalOutput").ap()

    with tile.TileContext(nc) as tc:
        with (
            tc.tile_pool(name="xpool", bufs=1) as xpool,
            tc.tile_pool(name="wpool", bufs=1) as wpool,
            tc.tile_pool(name="opool", bufs=4) as opool,
            tc.tile_pool(name="pspool", bufs=6, space="PSUM") as pspool,
        ):
            xr = [xpool.tile([96, W, B], f16, tag=f"x{i}", name=f"x{i}")
                  for i in range(RPC)]
            w0t = wpool.tile([96, OW, KW, C_OUT], f8, tag="w0t")
            w1t = wpool.tile([96, OW, KW, C_OUT], f8, tag="w1t")
            w23 = wpool.tile([96, 2, OW, KW, C_OUT], f8, tag="w23")
            biast = wpool.tile([128, RPC, GRPS, NGRP], f16, tag="biast")

            # 11 DMAs against a ~8-deep DMA-completion-sem pool: a recycled
            # sem makes every reader of the OLD owner's data wait for the
            # NEW owner's transfer (a 14-DMA variant chained rows 2-3's
            # bias matmuls behind rows 0-1's output writes). Emission order
            # ensures only the late out DMAs recycle sems (of selbias/x0/
            # x1, whose readers all executed long before) - harmless.
            # Tiny selector+bias first, then bulk in consumption order:
            # x per-row on scalar, w row-pairs on sync. The sync ring
            # drains first so the per-row output DMAs appended to it start
            # almost immediately once each row's copies finish.
            # x0..x3 emitted first so their sems occupy pool slots 1-4 and
            # each late out_i DMA recycles exactly x_i's sem (out_i is
            # issued after row i's copies, which strictly follow every
            # reader of x_i - recycling is safe by construction). Weights
            # split {w0, w1, w23} so row 0's matmuls gate on 0.59MB, not
            # on a 1.18MB row-pair: the PE starts ~6us earlier.
            for i in range(RPC):
                nc.scalar.dma_start(xr[i][:], xs_in[:, i])
            nc.sync.dma_start(biast[:], bias_in[:])
            nc.sync.dma_start(w0t[:], w_in[:, 0])
            nc.sync.dma_start(w1t[:], w_in[:, 1])
            nc.sync.dma_start(w23[:], w_in[:, 2:4])

            for i in range(RPC):
                orow = opool.tile([128, NPAIR, B], f16, tag=f"o{i}",
                                  name=f"o{i}")
                xt = xr[i]
                for g in range(GRPS):
                    ps = pspool.tile([128, NGRP, B], f32)
                    for pig in range(NGRP):
                        for par in range(2):
                            j = 2 * (NGRP * g + pig) + par
                            pslice = ps[64 * par:64 * par + 64, pig, :]
                            tp = (0, 64 * par)
                            if i == 0:
                                wl = [w0t[:, j, dj, :] for dj in range(KW)]
                            elif i == 1:
                                wl = [w1t[:, j, dj, :] for dj in range(KW)]
                            else:
                                wl = [w23[:, i - 2, j, dj, :]
                                      for dj in range(KW)]
                            nc.tensor.matmul(pslice, wl[0],
                                             xt[:, j, :],
                                             start=True, stop=False,
                                             tile_position=tp)
                            nc.tensor.matmul(pslice, wl[1],
                                             xt[:, j + 1, :],
                                             start=False, stop=False,
                                             tile_position=tp)
                            nc.tensor.matmul(pslice, wl[2],
                                             xt[:, j + 2, :],
                                             start=False, stop=True,
                                             tile_position=tp)
                    # bias enters here, broadcast over b - off the PE's
                    # critical path (the PE paces the rows; 32 selector
                    # matmuls cost it ~13us in the previous revision).
                    dst = orow[:, NGRP * g:NGRP * (g + 1), :]
                    bias_ap = biast[:, i, g, :].unsqueeze(2).broadcast_to(
                        [128, NGRP, B])
                    nc.vector.tensor_add(dst, ps[:], bias_ap)
                nc.sync.dma_start(out[:, i], orow[:])

    nc.compile()
    return nc


def _host_prep(x, weight, bias):
    """Full fp32 inputs -> list of per-core input dicts."""
    import ml_dtypes
    f8 = ml_dtypes.float8_e4m3

    # x: (B, C, H, W) -> (C, H, W, B) fp16
    x_t = np.ascontiguousarray(x.transpose(1, 2, 3, 0)).astype(np.float16)
    # w: (O, C, I, J, K) with K=(di*3+dj) -> [(di*32+c)=96, I, J, dj, O]
    # unscaled: the bias-add happens post-accumulation, so psum must hold
    # the true (unscaled) partial sums. Unscaled fp8 w measures 1.69e-2.
    w_r = weight.reshape(C_OUT, C_IN, OH, OW, KH, KW)
    w_t = w_r.transpose(4, 1, 2, 3, 5, 0)          # (di, c, I, J, dj, O)
    w_full = w_t.reshape(96, OH, OW, KW, C_OUT).astype(f8)
    # bias: (O, I, J) with j = 8g + 2k + par -> [(par*64+o), I, g, k]
    b_t = bias.reshape(C_OUT, OH, GRPS, NGRP, 2)   # (o, I, g, k, par)
    b_t = np.ascontiguousarray(b_t.transpose(4, 0, 1, 2, 3))  # (par,o,I,g,k)
    b_t = b_t.reshape(128, OH, GRPS, NGRP).astype(np.float16)

    in_maps = []
    for m in range(N_CORES):
        r0 = m * RPC
        xs = np.empty((96, RPC, W, B), np.float16)
        xsv = xs.reshape(KH, C_IN, RPC, W, B)
        for di in range(KH):
            xsv[di] = x_t[:, r0 + di:r0 + di + RPC]
        in_maps.append({
            "xs": xs,
            "w": np.ascontiguousarray(w_full[:, r0:r0 + RPC]),
            "bias2": np.ascontiguousarray(b_t[:, r0:r0 + RPC]),
        })
    return in_maps


def _gather(results):
    out_full = np.empty((B, C_OUT, OH, OW), np.float32)
    for m in range(N_CORES):
        r = results[m]["out"].astype(np.float32)          # (128, 4, 16, 128)
        r = r.reshape(2, C_OUT, RPC, NPAIR, B)            # par,o,i,jh,b
        r = r.transpose(4, 1, 2, 3, 0)                    # b,o,i,jh,par
        out_full[:, :, m * RPC:(m + 1) * RPC, :] = r.reshape(B, C_OUT, RPC, OW)
    return out_full


def kernel(x, weight, bias, _trace=False):
    from concourse.bass_utils import run_bass_kernel_spmd

    if "nc" not in _prog_cache:
        _prog_cache["nc"] = _build_program()
    nc = _prog_cache["nc"]

    in_maps = _host_prep(np.asarray(x), np.asarray(weight), np.asarray(bias))
    res = run_bass_kernel_spmd(nc, in_maps, core_ids=list(range(N_CORES)),
                               trace=_trace)
    out = _gather(res.results)
    if _trace:
        _prog_cache["last_result"] = res
    return out
